# revision 31
# baseline (speedup 1.0000x reference)
"""Trainium2 Bass kernel for the ConvLayer GNN message-passing problem.

Shapes (hardcoded): B=8, N=1024, T=8, C_in=C_out=64.
Sharding: batch B across the 8 NeuronCores (1 batch per core); k_tensor
replicated; BatchNorm stats AllReduce'd across cores.

Math (per batch b):
  G[n,m]   = exp(-dist[n,m]/2) = exp(PP[n,m] - sq[n]/2 - sq[m]/2)   (symmetric)
  rowsum[m]= sum_n G[m,n]
  q_t[n,m] = A[n,t] * G[n,m] * S[m,t] with
             A[n,t] = exp(ptd[t,n] - tsq[t]/2),  S[m,t] = exp(-ptd[t,m])
  out[n,o] = sum_t A[n,t] * sum_m G[n,m] * (S[m,t]/rowsum[m]) * (F @ K2)[m,(t,o)]
followed by cross-batch BatchNorm (training stats) + ReLU.

Layout notes:
  - dist is computed with a single f32r matmul using 4 contraction rows
    [px,py,pz,1] x [px,py,pz,-sq/2]; the per-row -sq[n]/2 comes in through
    the exp bias.
  - inputs are packed host-side into 3 DMA-able tensors (aux2/aux/pk) to
    amortize the ~625ns-per-DMA HWDGE descriptor cost.
  - out and out^2 live side by side per n-tile in one SBUF tile so BN
    stats need one 128-col matmul per n-tile, and the BN apply + store is
    a single strided 3-op DVE chain plus ONE output DMA.
"""

import ml_dtypes
import numpy as np

B, N, T, C = 8, 1024, 8, 64
NT = N // 128  # 8 n-tiles / m-tiles of 128
J = T * C  # 512, flattened (t, o)
BN_EPS = 1e-5
AUX2_W = 1032

_CACHE: dict = {}


def _build(reps=1, coll=True):
    import concourse.bacc as bacc
    import concourse.mybir as mybir
    import concourse.tile as tile

    f32 = mybir.dt.float32
    f32r = mybir.dt.float32r
    bf16 = mybir.dt.bfloat16
    Act = mybir.ActivationFunctionType
    Alu = mybir.AluOpType
    X = mybir.AxisListType.X

    nc = bacc.Bacc(
        "TRN2",
        target_bir_lowering=False,
        debug=False,
        enable_asserts=False,
        num_devices=B,
    )

    # ---- DRAM I/O (per-core shards supplied via in_maps) ----
    # dab rows 0-3: cols 0:1024 = [px,py,pz,1] (dist lhsT), cols
    # 1024:1032 = trT (rows 0-2), cols 1032:2056 = [px,py,pz,-sq/2]
    # (dist rhs).
    d_dab = nc.dram_tensor("dab", [4, 2056], f32r, kind="ExternalInput")
    # aux cols: 0:8 = sqmh[p,k] = -0.5*sq[128k+p]; 8:16 = etsq bcast;
    # 16:80 = gamma bcast; 80:144 = beta bcast.
    d_aux = nc.dram_tensor("aux", [128, 144], f32, kind="ExternalInput")
    # pk: [:, 0:1024] = funcT; [:, 1024:1536] = k2[c,(t,o)].
    d_pk = nc.dram_tensor("pk", [C, 1536], bf16, kind="ExternalInput")
    # out[p, (n, o)] — host transposes to [n*128+p, o].
    d_out = nc.dram_tensor("out", [128, NT * C], bf16, kind="ExternalOutput")

    with tile.TileContext(nc) as tc:
        with (
            nc.allow_low_precision("output tolerance is 2e-2; bf16 keeps 4e-3"),
            tc.tile_pool(name="cons", bufs=1) as cons,
            tc.tile_pool(name="gpool", bufs=NT) as gpool,
            tc.tile_pool(name="bpool", bufs=NT) as bpool,
            tc.tile_pool(name="work", bufs=2) as work,
            tc.tile_pool(name="psd", bufs=2, space="PSUM") as psd,
            tc.tile_pool(name="psf", bufs=1, space="PSUM") as psf,
            tc.tile_pool(name="psy", bufs=3, space="PSUM") as psy,
            tc.tile_pool(name="dram", bufs=1, space="DRAM") as dram,
        ):
          for _rep in range(max(1, reps)):
                # ---- packed input DMAs ----
                t_dab = cons.tile([4, 2056], f32r, tag="dab")
                t_aux = cons.tile([128, 144], f32, tag="aux")
                t_pk = cons.tile([C, 1536], bf16, tag="pk")
                nc.sync.dma_start(t_dab[:], d_dab.ap())
                nc.sync.dma_start(t_aux[:], d_aux.ap())
                nc.sync.dma_start(t_pk[:], d_pk.ap())

                t_ones_col = cons.tile([128, 1], bf16, tag="ones_col")
                nc.vector.memset(t_ones_col[:], 1.0)

                # ---- ptd matmuls: ps_ptd[p, 8k+t] = ptd[t, 128k+p] ----
                ps_ptd = psf.tile([128, NT * T], f32, tag="f")
                for k in range(NT):
                    nc.tensor.matmul(
                        ps_ptd[:, k * T : (k + 1) * T],
                        t_dab[0:3, k * 128 : (k + 1) * 128],
                        t_dab[0:3, 1024:1032],
                        start=True,
                        stop=True,
                    )
                # S = exp(-ptd), A = exp(ptd) * etsq
                t_S = cons.tile([128, NT * T], f32, tag="S")
                t_A = cons.tile([128, NT * T], f32, tag="A")
                nc.scalar.activation(t_S[:], ps_ptd[:], Act.Exp, scale=-1.0)
                nc.scalar.activation(t_A[:], ps_ptd[:], Act.Exp, scale=1.0)
                nc.vector.tensor_mul(
                    t_A[:].rearrange("p (k t) -> p k t", k=NT),
                    t_A[:].rearrange("p (k t) -> p k t", k=NT),
                    t_aux[:, 8:16].unsqueeze(1).broadcast_to([128, NT, T]),
                )

                t_rs = cons.tile([128, NT], f32, tag="rs")  # rowsum
                nc.vector.memset(t_rs[:], 0.0)  # HW accum_out accumulates
                t_rcp = cons.tile([128, NT], f32, tag="rcp")  # 1/rowsum

                # ---- k-loop: dist matmul + exp + FK2 + B~ ----
                g_tiles = []
                b_tiles = []
                for k in range(NT):
                    ps_d = psd.tile([128, N], f32, tag="dist")
                    nc.tensor.matmul(
                        ps_d[:, 0:512],
                        t_dab[0:4, k * 128 : (k + 1) * 128],
                        t_dab[0:4, 1032:1544],
                        start=True,
                        stop=True,
                    )
                    nc.tensor.matmul(
                        ps_d[:, 512:1024],
                        t_dab[0:4, k * 128 : (k + 1) * 128],
                        t_dab[0:4, 1544:2056],
                        start=True,
                        stop=True,
                    )
                    t_g = gpool.tile([128, N], f32r, tag="g")
                    nc.scalar.activation(
                        t_g[:],
                        ps_d[:],
                        Act.Exp,
                        bias=t_aux[:, k : k + 1],
                        scale=1.0,
                        accum_out=t_rs[:, k : k + 1],
                    )
                    g_tiles.append(t_g)

                    nc.vector.reciprocal(t_rcp[:, k : k + 1], t_rs[:, k : k + 1])
                    ps_f = psf.tile([128, J], f32, tag="f")
                    nc.tensor.matmul(
                        ps_f[:],
                        t_pk[:, k * 128 : (k + 1) * 128],
                        t_pk[:, 1024:1536],
                        start=True,
                        stop=True,
                    )
                    # B~[p,(t,o)] = (S[p,t] * rcp[p]) * FK2[p,(t,o)]
                    t_b = bpool.tile([128, J], f32r, tag="b")
                    nc.vector.scalar_tensor_tensor(
                        t_b[:].rearrange("p (t o) -> p t o", t=T),
                        t_S[:, k * T : (k + 1) * T].unsqueeze(2).broadcast_to(
                            [128, T, C]
                        ),
                        t_rcp[:, k : k + 1],
                        ps_f[:].rearrange("p (t o) -> p t o", t=T),
                        op0=Alu.mult,
                        op1=Alu.mult,
                    )
                    b_tiles.append(t_b)

                # warm the sqrt activation table after the exps are done
                t_scr = cons.tile([1, 1], f32, tag="scr")
                nc.scalar.activation(t_scr[:], t_rs[0:1, 0:1], Act.Sqrt)

                # ---- n-loop: y matmuls + t-contraction ----
                # bf16 out tiles: rel tol is 2e-2, bf16 keeps ~4e-3; DVE gets
                # its 2x/4x 16-bit modes for the BN apply and the store halves
                out_all = cons.tile([128, NT * C], bf16, tag="out_all")
                sq_all = cons.tile([128, NT * C], bf16, tag="sq_all")
                for n in range(NT):
                    ps_y = psy.tile([128, J], f32, tag="y")
                    for k in range(NT):
                        nc.tensor.matmul(
                            ps_y[:],
                            g_tiles[k][:, n * 128 : (n + 1) * 128],
                            b_tiles[k][:],
                            start=(k == 0),
                            stop=(k == NT - 1),
                        )
                    t_m = work.tile([128, J], f32, tag="m")
                    # DVE (GPSIMD cannot read PSUM)
                    nc.vector.tensor_mul(
                        t_m[:].rearrange("p (t o) -> p t o", t=T),
                        ps_y[:].rearrange("p (t o) -> p t o", t=T),
                        t_A[:, n * T : (n + 1) * T].unsqueeze(2).broadcast_to(
                            [128, T, C]
                        ),
                    )
                    t_out = out_all[:, n * C : (n + 1) * C]
                    nc.vector.tensor_reduce(
                        t_out,
                        t_m[:].rearrange("p (t o) -> p o t", t=T),
                        axis=X,
                        op=Alu.add,
                    )
                    if n == NT - 1:
                        nc.vector.tensor_mul(
                            sq_all[:, n * C : (n + 1) * C], t_out, t_out
                        )
                    else:
                        nc.scalar.activation(
                            sq_all[:, n * C : (n + 1) * C], t_out, Act.Square
                        )

                # ---- stats: colsum matmuls + n-folds ----
                ps_s1 = psy.tile([1, J], f32, tag="y")
                ps_s2 = psy.tile([1, J], f32, tag="y")
                nc.tensor.matmul(
                    ps_s1[:], t_ones_col[:], out_all[:], start=True, stop=True
                )
                nc.tensor.matmul(
                    ps_s2[:], t_ones_col[:], sq_all[:], start=True, stop=True
                )
                t_stat_sb = cons.tile([1, 2 * C], f32, tag="stat_sb")
                nc.vector.tensor_reduce(
                    t_stat_sb[:, 0:C],
                    ps_s1[:].rearrange("p (n o) -> p o n", n=NT),
                    axis=X,
                    op=Alu.add,
                )
                nc.vector.tensor_reduce(
                    t_stat_sb[:, C : 2 * C],
                    ps_s2[:].rearrange("p (n o) -> p o n", n=NT),
                    axis=X,
                    op=Alu.add,
                )

                # ---- BN stats AllReduce + scale/shift ----
                ib = dram.tile([1, 2 * C], f32, tag="ib")
                ob = dram.tile([1, 2 * C], f32, tag="ob")
                nc.sync.dma_start(ib[:, 0:C], t_stat_sb[:, 0:C])
                nc.sync.dma_start(ib[:, C : 2 * C], t_stat_sb[:, C : 2 * C])
                if coll:
                    nc.gpsimd.collective_compute(
                        "AllReduce",
                        Alu.add,
                        replica_groups=[list(range(B))],
                        ins=[ib.opt()],
                        outs=[ob.opt()],
                    )
                else:
                    nc.sync.dma_start(ob[:], ib[:])
                # partition-broadcast the summed stats to all 128 partitions
                t_sums = cons.tile([128, 2 * C], f32, tag="sums")
                nc.sync.dma_start(
                    t_sums[:], ob[:].broadcast_to([128, 2 * C])
                )

                # scale = gamma * rsqrt(var + eps), shift = beta - mean*scale
                # with var = (s2 - s1^2/BN)/BN, mean = s1/BN, from raw sums.
                rbn = 1.0 / (B * N)
                s1 = t_sums[:, 0:C]
                s2 = t_sums[:, C : 2 * C]
                t_eps = cons.tile([128, 1], f32, tag="eps")
                nc.vector.memset(t_eps[:], BN_EPS)
                t_w1 = cons.tile([128, C], f32, tag="w1")
                nc.vector.tensor_mul(t_w1[:], s1, s1)  # s1^2
                t_var = cons.tile([128, C], f32, tag="var")
                nc.vector.scalar_tensor_tensor(
                    t_var[:], t_w1[:], -rbn, s2, op0=Alu.mult, op1=Alu.add
                )  # s2 - s1^2/BN
                t_std = cons.tile([128, C], f32, tag="std")
                nc.scalar.activation(
                    t_std[:], t_var[:], Act.Sqrt, bias=t_eps[:], scale=rbn
                )
                t_ss = cons.tile([128, 2 * C], bf16, tag="ss")
                t_rstd = cons.tile([128, C], f32, tag="rstd")
                nc.vector.reciprocal(t_rstd[:], t_std[:])
                gamma = t_aux[:, 16:80]
                beta = t_aux[:, 80:144]
                nc.vector.tensor_mul(t_ss[:, 0:C], t_rstd[:], gamma)  # scale
                nc.vector.tensor_mul(t_w1[:], s1, t_ss[:, 0:C])  # s1*scale
                nc.vector.scalar_tensor_tensor(
                    t_ss[:, C : 2 * C], t_w1[:], -rbn, beta,
                    op0=Alu.mult, op1=Alu.add,
                )  # beta - mean*scale

                # ---- apply BN + ReLU, single contiguous store ----
                t_fin = cons.tile([128, NT * C], bf16, tag="fin")
                out_v = out_all[:].rearrange("p (n o) -> p n o", n=NT)
                fin_v = t_fin[:].rearrange("p (n o) -> p n o", n=NT)
                nc.vector.tensor_mul(
                    fin_v,
                    out_v,
                    t_ss[:, 0:C].unsqueeze(1).broadcast_to([128, NT, C]),
                )
                nc.vector.tensor_add(
                    fin_v,
                    fin_v,
                    t_ss[:, C : 2 * C].unsqueeze(1).broadcast_to([128, NT, C]),
                )
                nc.vector.tensor_scalar_max(t_fin[:], t_fin[:], 0.0)
                nc.sync.dma_start(d_out.ap(), t_fin[:])

    nc.compile()
    return nc


def _prep_inputs(points, translations, functions, k_tensor, gamma, beta):
    """Host-side sharding + layout prep (O(N) work only)."""
    k2 = np.ascontiguousarray(
        np.transpose(k_tensor.astype(np.float32), (1, 2, 0)).reshape(C, J)
    )
    g32 = gamma.astype(np.float32)
    b32 = beta.astype(np.float32)
    in_maps = []
    for i in range(B):
        pts = points[i].astype(np.float32)  # [N, 3]
        sq = (pts * pts).sum(axis=1)  # [N]
        tr = translations[i].astype(np.float32)  # [T, 3]
        tsq = (tr * tr).sum(axis=1)  # [T]

        dab = np.zeros((4, 2056), dtype=np.float32)
        dab[0:3, 0:N] = pts.T
        dab[3, 0:N] = 1.0
        dab[0:3, 1024:1032] = tr.T
        dab[0:3, 1032 : 1032 + N] = pts.T
        dab[3, 1032 : 1032 + N] = -0.5 * sq

        aux = np.empty((128, 144), dtype=np.float32)
        aux[:, 0:8] = (-0.5 * sq).reshape(NT, 128).T
        aux[:, 8:16] = np.exp(-0.5 * tsq)[None, :]
        aux[:, 16:80] = g32[None, :]
        aux[:, 80:144] = b32[None, :]

        pk = np.empty((C, 1536), dtype=ml_dtypes.bfloat16)
        pk[:, 0:1024] = functions[i].astype(np.float32).T
        pk[:, 1024:1536] = k2

        in_maps.append(
            {
                "dab": np.ascontiguousarray(dab),
                "aux": np.ascontiguousarray(aux),
                "pk": np.ascontiguousarray(pk),
            }
        )
    return in_maps


def kernel(points, translations, functions, k_tensor, gamma, beta):
    from concourse import bass_utils

    if "nc" not in _CACHE:
        _CACHE["nc"] = _build()
    nc = _CACHE["nc"]

    in_maps = _prep_inputs(points, translations, functions, k_tensor, gamma, beta)
    res = bass_utils.run_bass_kernel_spmd(nc, in_maps, core_ids=list(range(B)))
    # device layout is [p, (n, o)]; row n*128+p of the real output
    out = np.stack(
        [
            res.results[i]["out"]
            .astype(np.float32)
            .reshape(128, NT, C)
            .transpose(1, 0, 2)
            .reshape(N, C)
            for i in range(B)
        ],
        axis=0,
    )
    return out.astype(np.float32)


# revision 35
# speedup vs baseline: 1.0372x; 1.0372x over previous
"""Trainium2 Bass kernel for the ConvLayer GNN message-passing problem.

Shapes (hardcoded): B=8, N=1024, T=8, C_in=C_out=64.
Sharding: batch B across the 8 NeuronCores (1 batch per core); k_tensor
replicated; BatchNorm stats AllReduce'd across cores.

Math (per batch b):
  G[n,m]   = exp(-dist[n,m]/2) = exp(PP[n,m] - sq[n]/2 - sq[m]/2)   (symmetric)
  rowsum[m]= sum_n G[m,n]
  q_t[n,m] = A[n,t] * G[n,m] * S[m,t] with
             A[n,t] = exp(ptd[t,n] - tsq[t]/2),  S[m,t] = exp(-ptd[t,m])
  out[n,o] = sum_t A[n,t] * sum_m G[n,m] * (S[m,t]/rowsum[m]) * (F @ K2)[m,(t,o)]
followed by cross-batch BatchNorm (training stats) + ReLU.

Layout notes:
  - dist is computed with f32r matmuls over 4 contraction rows
    [px,py,pz,1] x [px,py,pz,-sq/2]; the per-row -sq[n]/2 comes in through
    the exp bias (f32r runs at bf16 col rate for >=256-col outputs).
  - inputs are packed host-side into 3 DMA tensors (dab/aux/pk) to
    amortize the ~625ns-per-DMA HWDGE descriptor cost; pk is bf16.
  - B~ is one fused scalar_tensor_tensor; stats are two post-loop colsum
    matmuls + folds (keeping PE's in-order stream stall-free); BN apply is
    a 3-op bf16 DVE chain + ONE contiguous store (host undoes the [p,n,o]
    transpose).
  - BN stats cross-core exchange: AllGather + local fold (AllReduce plus a
    partition-broadcast DMA return measured ~70us/rep slower on HW).
"""

import ml_dtypes
import numpy as np

B, N, T, C = 8, 1024, 8, 64
NT = N // 128  # 8 n-tiles / m-tiles of 128
J = T * C  # 512, flattened (t, o)
BN_EPS = 1e-5
AUX2_W = 1032

_CACHE: dict = {}


def _build(reps=1, coll="gather"):
    import concourse.bacc as bacc
    import concourse.mybir as mybir
    import concourse.tile as tile

    f32 = mybir.dt.float32
    f32r = mybir.dt.float32r
    bf16 = mybir.dt.bfloat16
    Act = mybir.ActivationFunctionType
    Alu = mybir.AluOpType
    X = mybir.AxisListType.X

    nc = bacc.Bacc(
        "TRN2",
        target_bir_lowering=False,
        debug=False,
        enable_asserts=False,
        num_devices=B,
    )

    # ---- DRAM I/O (per-core shards supplied via in_maps) ----
    # dab rows 0-3: cols 0:1024 = [px,py,pz,1] (dist lhsT), cols
    # 1024:1032 = trT (rows 0-2), cols 1032:2056 = [px,py,pz,-sq/2]
    # (dist rhs).
    d_dab = nc.dram_tensor("dab", [4, 2056], f32r, kind="ExternalInput")
    # aux cols: 0:8 = sqmh[p,k] = -0.5*sq[128k+p]; 8:16 = etsq bcast;
    # 16:80 = gamma bcast; 80:144 = beta bcast.
    d_aux = nc.dram_tensor("aux", [128, 144], f32, kind="ExternalInput")
    # pk: [:, 0:1024] = funcT; [:, 1024:1536] = k2[c,(t,o)].
    d_pk = nc.dram_tensor("pk", [C, 1536], bf16, kind="ExternalInput")
    # out[p, (n, o)] — host transposes to [n*128+p, o].
    d_out = nc.dram_tensor("out", [128, NT * C], bf16, kind="ExternalOutput")

    with tile.TileContext(nc) as tc:
        with (
            nc.allow_low_precision("output tolerance is 2e-2; bf16 keeps 4e-3"),
            tc.tile_pool(name="cons", bufs=1) as cons,
            tc.tile_pool(name="gpool", bufs=NT) as gpool,
            tc.tile_pool(name="bpool", bufs=NT) as bpool,
            tc.tile_pool(name="work", bufs=2) as work,
            tc.tile_pool(name="psd", bufs=2, space="PSUM") as psd,
            tc.tile_pool(name="psf", bufs=1, space="PSUM") as psf,
            tc.tile_pool(name="psy", bufs=3, space="PSUM") as psy,
            tc.tile_pool(name="dram", bufs=1, space="DRAM") as dram,
        ):
          for _rep in range(max(1, reps)):
                # ---- packed input DMAs ----
                t_dab = cons.tile([4, 2056], f32r, tag="dab")
                t_aux = cons.tile([128, 144], f32, tag="aux")
                t_pk = cons.tile([C, 1536], bf16, tag="pk")
                nc.sync.dma_start(t_dab[:], d_dab.ap())
                nc.sync.dma_start(t_aux[:], d_aux.ap())
                nc.sync.dma_start(t_pk[:], d_pk.ap())

                t_ones_col = cons.tile([128, 1], bf16, tag="ones_col")
                nc.vector.memset(t_ones_col[:], 1.0)

                # ---- ptd matmuls: ps_ptd[p, 8k+t] = ptd[t, 128k+p] ----
                ps_ptd = psf.tile([128, NT * T], f32, tag="f")
                for k in range(NT):
                    nc.tensor.matmul(
                        ps_ptd[:, k * T : (k + 1) * T],
                        t_dab[0:3, k * 128 : (k + 1) * 128],
                        t_dab[0:3, 1024:1032],
                        start=True,
                        stop=True,
                    )
                # S = exp(-ptd), A = exp(ptd) * etsq
                t_S = cons.tile([128, NT * T], f32, tag="S")
                t_A = cons.tile([128, NT * T], f32, tag="A")
                nc.scalar.activation(t_S[:], ps_ptd[:], Act.Exp, scale=-1.0)
                nc.scalar.activation(t_A[:], ps_ptd[:], Act.Exp, scale=1.0)
                nc.vector.tensor_mul(
                    t_A[:].rearrange("p (k t) -> p k t", k=NT),
                    t_A[:].rearrange("p (k t) -> p k t", k=NT),
                    t_aux[:, 8:16].unsqueeze(1).broadcast_to([128, NT, T]),
                )

                t_rs = cons.tile([128, NT], f32, tag="rs")  # rowsum
                nc.vector.memset(t_rs[:], 0.0)  # HW accum_out accumulates
                t_rcp = cons.tile([128, NT], f32, tag="rcp")  # 1/rowsum

                # ---- k-loop: dist matmul + exp + FK2 + B~ ----
                g_tiles = []
                b_tiles = []
                for k in range(NT):
                    ps_d = psd.tile([128, N], f32, tag="dist")
                    nc.tensor.matmul(
                        ps_d[:, 0:512],
                        t_dab[0:4, k * 128 : (k + 1) * 128],
                        t_dab[0:4, 1032:1544],
                        start=True,
                        stop=True,
                    )
                    nc.tensor.matmul(
                        ps_d[:, 512:1024],
                        t_dab[0:4, k * 128 : (k + 1) * 128],
                        t_dab[0:4, 1544:2056],
                        start=True,
                        stop=True,
                    )
                    t_g = gpool.tile([128, N], f32r, tag="g")
                    nc.scalar.activation(
                        t_g[:],
                        ps_d[:],
                        Act.Exp,
                        bias=t_aux[:, k : k + 1],
                        scale=1.0,
                        accum_out=t_rs[:, k : k + 1],
                    )
                    g_tiles.append(t_g)

                    nc.vector.reciprocal(t_rcp[:, k : k + 1], t_rs[:, k : k + 1])
                    ps_f = psf.tile([128, J], f32, tag="f")
                    nc.tensor.matmul(
                        ps_f[:],
                        t_pk[:, k * 128 : (k + 1) * 128],
                        t_pk[:, 1024:1536],
                        start=True,
                        stop=True,
                    )
                    # B~[p,(t,o)] = (S[p,t] * rcp[p]) * FK2[p,(t,o)]
                    t_b = bpool.tile([128, J], f32r, tag="b")
                    nc.vector.scalar_tensor_tensor(
                        t_b[:].rearrange("p (t o) -> p t o", t=T),
                        t_S[:, k * T : (k + 1) * T].unsqueeze(2).broadcast_to(
                            [128, T, C]
                        ),
                        t_rcp[:, k : k + 1],
                        ps_f[:].rearrange("p (t o) -> p t o", t=T),
                        op0=Alu.mult,
                        op1=Alu.mult,
                    )
                    b_tiles.append(t_b)

                # warm the sqrt activation table after the exps are done
                t_scr = cons.tile([1, 1], f32, tag="scr")
                nc.scalar.activation(t_scr[:], t_rs[0:1, 0:1], Act.Sqrt)

                # ---- n-loop: y matmuls + t-contraction ----
                # bf16 out tiles: rel tol is 2e-2, bf16 keeps ~4e-3; DVE gets
                # its 2x/4x 16-bit modes for the BN apply and the store halves
                out_all = cons.tile([128, NT * C], bf16, tag="out_all")
                sq_all = cons.tile([128, NT * C], bf16, tag="sq_all")
                for n in range(NT):
                    ps_y = psy.tile([128, J], f32, tag="y")
                    for k in range(NT):
                        nc.tensor.matmul(
                            ps_y[:],
                            g_tiles[k][:, n * 128 : (n + 1) * 128],
                            b_tiles[k][:],
                            start=(k == 0),
                            stop=(k == NT - 1),
                        )
                    t_m = work.tile([128, J], f32, tag="m")
                    # DVE (GPSIMD cannot read PSUM)
                    nc.vector.tensor_mul(
                        t_m[:].rearrange("p (t o) -> p t o", t=T),
                        ps_y[:].rearrange("p (t o) -> p t o", t=T),
                        t_A[:, n * T : (n + 1) * T].unsqueeze(2).broadcast_to(
                            [128, T, C]
                        ),
                    )
                    t_out = out_all[:, n * C : (n + 1) * C]
                    nc.vector.tensor_reduce(
                        t_out,
                        t_m[:].rearrange("p (t o) -> p o t", t=T),
                        axis=X,
                        op=Alu.add,
                    )
                    if n == NT - 1:
                        nc.vector.tensor_mul(
                            sq_all[:, n * C : (n + 1) * C], t_out, t_out
                        )
                    else:
                        nc.scalar.activation(
                            sq_all[:, n * C : (n + 1) * C], t_out, Act.Square
                        )

                # ---- stats: colsum matmuls + n-folds ----
                ps_s1 = psy.tile([1, J], f32, tag="y")
                ps_s2 = psy.tile([1, J], f32, tag="y")
                nc.tensor.matmul(
                    ps_s1[:], t_ones_col[:], out_all[:], start=True, stop=True
                )
                nc.tensor.matmul(
                    ps_s2[:], t_ones_col[:], sq_all[:], start=True, stop=True
                )
                t_stat_sb = cons.tile([1, 2 * C], f32, tag="stat_sb")
                nc.vector.tensor_reduce(
                    t_stat_sb[:, 0:C],
                    ps_s1[:].rearrange("p (n o) -> p o n", n=NT),
                    axis=X,
                    op=Alu.add,
                )
                nc.vector.tensor_reduce(
                    t_stat_sb[:, C : 2 * C],
                    ps_s2[:].rearrange("p (n o) -> p o n", n=NT),
                    axis=X,
                    op=Alu.add,
                )

                # ---- BN stats collective + scale/shift ----
                # coll: "gather" (default) = AllGather + local fold;
                # "reduce" = AllReduce + partition-broadcast DMA;
                # False = fake passthrough (single-core timing sim)
                ib = dram.tile([1, 2 * C], f32, tag="ib")
                nc.sync.dma_start(ib[:, 0:C], t_stat_sb[:, 0:C])
                nc.sync.dma_start(ib[:, C : 2 * C], t_stat_sb[:, C : 2 * C])
                P = 1 if coll == "gather" else 128
                t_sums = cons.tile([P, 2 * C], f32, tag="sums")
                if coll == "gather":
                    ob8 = dram.tile([B, 2 * C], f32, tag="ob8")
                    nc.gpsimd.collective_compute(
                        "AllGather",
                        Alu.bypass,
                        replica_groups=[list(range(B))],
                        ins=[ib.opt()],
                        outs=[ob8.opt()],
                    )
                    t_g8 = cons.tile([1, B * 2 * C], f32, tag="g8")
                    nc.sync.dma_start(
                        t_g8[:],
                        ob8[:].rearrange("r x -> (r x)").unsqueeze(0),
                    )
                    nc.vector.tensor_reduce(
                        t_sums[:],
                        t_g8[:].rearrange("p (r x) -> p x r", r=B),
                        axis=X,
                        op=Alu.add,
                    )
                else:
                    ob = dram.tile([1, 2 * C], f32, tag="ob")
                    if coll:
                        nc.gpsimd.collective_compute(
                            "AllReduce",
                            Alu.add,
                            replica_groups=[list(range(B))],
                            ins=[ib.opt()],
                            outs=[ob.opt()],
                        )
                    else:
                        nc.sync.dma_start(ob[:], ib[:])
                    # partition-broadcast summed stats to all 128 partitions
                    nc.sync.dma_start(
                        t_sums[:], ob[:].broadcast_to([128, 2 * C])
                    )

                # scale = gamma * rsqrt(var + eps), shift = beta - mean*scale
                # with var = (s2 - s1^2/BN)/BN, mean = s1/BN, from raw sums.
                rbn = 1.0 / (B * N)
                s1 = t_sums[:, 0:C]
                s2 = t_sums[:, C : 2 * C]
                t_eps = cons.tile([P, 1], f32, tag="eps")
                nc.vector.memset(t_eps[:], BN_EPS)
                t_w1 = cons.tile([P, C], f32, tag="w1")
                nc.vector.tensor_mul(t_w1[:], s1, s1)  # s1^2
                t_var = cons.tile([P, C], f32, tag="var")
                nc.vector.scalar_tensor_tensor(
                    t_var[:], t_w1[:], -rbn, s2, op0=Alu.mult, op1=Alu.add
                )  # s2 - s1^2/BN
                t_std = cons.tile([P, C], f32, tag="std")
                nc.scalar.activation(
                    t_std[:], t_var[:], Act.Sqrt, bias=t_eps[:], scale=rbn
                )
                t_ssw = cons.tile([P, 2 * C], f32, tag="ssw")
                t_rstd = cons.tile([P, C], f32, tag="rstd")
                nc.vector.reciprocal(t_rstd[:], t_std[:])
                gamma = t_aux[0:P, 16:80]
                beta = t_aux[0:P, 80:144]
                nc.vector.tensor_mul(t_ssw[:, 0:C], t_rstd[:], gamma)  # scale
                nc.vector.tensor_mul(t_w1[:], s1, t_ssw[:, 0:C])  # s1*scale
                nc.vector.scalar_tensor_tensor(
                    t_ssw[:, C : 2 * C], t_w1[:], -rbn, beta,
                    op0=Alu.mult, op1=Alu.add,
                )  # beta - mean*scale
                t_ss = cons.tile([128, 2 * C], bf16, tag="ss")
                if P == 1:
                    # broadcast scale|shift to all partitions via matmul
                    t_ones_row = cons.tile([1, 128], f32, tag="ones_row")
                    nc.vector.memset(t_ones_row[:], 1.0)
                    ps_bc = psy.tile([128, 2 * C], f32, tag="y")
                    nc.tensor.matmul(
                        ps_bc[:], t_ones_row[:], t_ssw[:], start=True,
                        stop=True,
                    )
                    nc.vector.tensor_copy(t_ss[:], ps_bc[:])
                else:
                    nc.vector.tensor_copy(t_ss[:], t_ssw[:])

                # ---- apply BN + ReLU, single contiguous store ----
                t_fin = cons.tile([128, NT * C], bf16, tag="fin")
                out_v = out_all[:].rearrange("p (n o) -> p n o", n=NT)
                fin_v = t_fin[:].rearrange("p (n o) -> p n o", n=NT)
                nc.vector.tensor_mul(
                    fin_v,
                    out_v,
                    t_ss[:, 0:C].unsqueeze(1).broadcast_to([128, NT, C]),
                )
                nc.vector.tensor_add(
                    fin_v,
                    fin_v,
                    t_ss[:, C : 2 * C].unsqueeze(1).broadcast_to([128, NT, C]),
                )
                nc.vector.tensor_scalar_max(t_fin[:], t_fin[:], 0.0)
                nc.sync.dma_start(d_out.ap(), t_fin[:])

    nc.compile()
    return nc


def _prep_inputs(points, translations, functions, k_tensor, gamma, beta):
    """Host-side sharding + layout prep (O(N) work only)."""
    k2 = np.ascontiguousarray(
        np.transpose(k_tensor.astype(np.float32), (1, 2, 0)).reshape(C, J)
    )
    g32 = gamma.astype(np.float32)
    b32 = beta.astype(np.float32)
    in_maps = []
    for i in range(B):
        pts = points[i].astype(np.float32)  # [N, 3]
        sq = (pts * pts).sum(axis=1)  # [N]
        tr = translations[i].astype(np.float32)  # [T, 3]
        tsq = (tr * tr).sum(axis=1)  # [T]

        dab = np.zeros((4, 2056), dtype=np.float32)
        dab[0:3, 0:N] = pts.T
        dab[3, 0:N] = 1.0
        dab[0:3, 1024:1032] = tr.T
        dab[0:3, 1032 : 1032 + N] = pts.T
        dab[3, 1032 : 1032 + N] = -0.5 * sq

        aux = np.empty((128, 144), dtype=np.float32)
        aux[:, 0:8] = (-0.5 * sq).reshape(NT, 128).T
        aux[:, 8:16] = np.exp(-0.5 * tsq)[None, :]
        aux[:, 16:80] = g32[None, :]
        aux[:, 80:144] = b32[None, :]

        pk = np.empty((C, 1536), dtype=ml_dtypes.bfloat16)
        pk[:, 0:1024] = functions[i].astype(np.float32).T
        pk[:, 1024:1536] = k2

        in_maps.append(
            {
                "dab": np.ascontiguousarray(dab),
                "aux": np.ascontiguousarray(aux),
                "pk": np.ascontiguousarray(pk),
            }
        )
    return in_maps


def kernel(points, translations, functions, k_tensor, gamma, beta):
    from concourse import bass_utils

    if "nc" not in _CACHE:
        _CACHE["nc"] = _build()
    nc = _CACHE["nc"]

    in_maps = _prep_inputs(points, translations, functions, k_tensor, gamma, beta)
    res = bass_utils.run_bass_kernel_spmd(nc, in_maps, core_ids=list(range(B)))
    # device layout is [p, (n, o)]; row n*128+p of the real output
    out = np.stack(
        [
            res.results[i]["out"]
            .astype(np.float32)
            .reshape(128, NT, C)
            .transpose(1, 0, 2)
            .reshape(N, C)
            for i in range(B)
        ],
        axis=0,
    )
    return out.astype(np.float32)


# revision 36
# speedup vs baseline: 1.5575x; 1.5016x over previous
"""Trainium2 Bass kernel for the ConvLayer GNN message-passing problem.

Shapes (hardcoded): B=8, N=1024, T=8, C_in=C_out=64.
Sharding: batch B across the 8 NeuronCores (1 batch per core); k_tensor
replicated; BatchNorm stats AllReduce'd across cores.

Math (per batch b):
  G[n,m]   = exp(-dist[n,m]/2) = exp(PP[n,m] - sq[n]/2 - sq[m]/2)   (symmetric)
  rowsum[m]= sum_n G[m,n]
  q_t[n,m] = A[n,t] * G[n,m] * S[m,t] with
             A[n,t] = exp(ptd[t,n] - tsq[t]/2),  S[m,t] = exp(-ptd[t,m])
  out[n,o] = sum_t A[n,t] * sum_m G[n,m] * (S[m,t]/rowsum[m]) * (F @ K2)[m,(t,o)]
followed by cross-batch BatchNorm (training stats) + ReLU.

Layout notes:
  - dist is computed with f32r matmuls over 4 contraction rows
    [px,py,pz,1] x [px,py,pz,-sq/2]; the per-row -sq[n]/2 comes in through
    the exp bias (f32r runs at bf16 col rate for >=256-col outputs).
  - inputs are packed host-side into 3 DMA tensors (dab/aux/pk) to
    amortize the ~625ns-per-DMA HWDGE descriptor cost; pk is bf16.
  - B~ is one fused scalar_tensor_tensor; stats are two post-loop colsum
    matmuls + folds (keeping PE's in-order stream stall-free); BN apply is
    a 3-op bf16 DVE chain + ONE contiguous store (host undoes the [p,n,o]
    transpose).
  - BN stats cross-core exchange: AllGather + local fold (AllReduce plus a
    partition-broadcast DMA return measured ~70us/rep slower on HW).
"""

import ml_dtypes
import numpy as np

B, N, T, C = 8, 1024, 8, 64
NT = N // 128  # 8 n-tiles / m-tiles of 128
J = T * C  # 512, flattened (t, o)
BN_EPS = 1e-5

_CACHE: dict = {}


def _build(reps=1, coll="gather"):
    import concourse.bacc as bacc
    import concourse.mybir as mybir
    import concourse.tile as tile

    f32 = mybir.dt.float32
    f32r = mybir.dt.float32r
    bf16 = mybir.dt.bfloat16
    Act = mybir.ActivationFunctionType
    Alu = mybir.AluOpType
    X = mybir.AxisListType.X

    nc = bacc.Bacc(
        "TRN2",
        target_bir_lowering=False,
        debug=False,
        enable_asserts=False,
        num_devices=B,
    )

    # ---- DRAM I/O (per-core shards supplied via in_maps) ----
    # dab rows 0-3: cols 0:1024 = [px,py,pz,1] (dist lhsT), cols
    # 1024:1032 = trT (rows 0-2), cols 1032:2056 = [px,py,pz,-sq/2]
    # (dist rhs).
    d_dab = nc.dram_tensor("dab", [4, 2056], f32r, kind="ExternalInput")
    # aux cols: 0:8 = sqmh[p,k] = -0.5*sq[128k+p]; 8:16 = etsq bcast;
    # 16:80 = gamma bcast; 80:144 = beta bcast.
    d_aux = nc.dram_tensor("aux", [128, 144], f32, kind="ExternalInput")
    # pk: [:, 0:1024] = funcT; [:, 1024:1536] = k2[c,(t,o)].
    d_pk = nc.dram_tensor("pk", [C, 1536], bf16, kind="ExternalInput")
    # out[p, (n, o)] — host transposes to [n*128+p, o].
    d_out = nc.dram_tensor("out", [128, NT * C], bf16, kind="ExternalOutput")

    with tile.TileContext(nc) as tc:
        with (
            nc.allow_low_precision("output tolerance is 2e-2; bf16 keeps 4e-3"),
            tc.tile_pool(name="cons", bufs=1) as cons,
            tc.tile_pool(name="gpool", bufs=NT) as gpool,
            tc.tile_pool(name="bpool", bufs=NT) as bpool,
            tc.tile_pool(name="work", bufs=2) as work,
            tc.tile_pool(name="psd", bufs=2, space="PSUM") as psd,
            tc.tile_pool(name="psf", bufs=1, space="PSUM") as psf,
            tc.tile_pool(name="psy", bufs=3, space="PSUM") as psy,
            tc.tile_pool(name="dram", bufs=1, space="DRAM") as dram,
        ):
          for _rep in range(max(1, reps)):
                # ---- packed input DMAs ----
                t_dab = cons.tile([4, 2056], f32r, tag="dab")
                t_aux = cons.tile([128, 144], f32, tag="aux")
                t_pk = cons.tile([C, 1536], bf16, tag="pk")
                nc.sync.dma_start(t_dab[:], d_dab.ap())
                nc.sync.dma_start(t_aux[:], d_aux.ap())
                nc.sync.dma_start(t_pk[:], d_pk.ap())

                t_ones_col = cons.tile([128, 1], bf16, tag="ones_col")
                nc.vector.memset(t_ones_col[:], 1.0)

                # ---- ptd matmuls: ps_ptd[p, 8k+t] = ptd[t, 128k+p] ----
                ps_ptd = psf.tile([128, NT * T], f32, tag="f")
                for k in range(NT):
                    nc.tensor.matmul(
                        ps_ptd[:, k * T : (k + 1) * T],
                        t_dab[0:3, k * 128 : (k + 1) * 128],
                        t_dab[0:3, 1024:1032],
                        start=True,
                        stop=True,
                    )
                # S = exp(-ptd), A = exp(ptd) * etsq
                t_S = cons.tile([128, NT * T], f32, tag="S")
                t_A = cons.tile([128, NT * T], f32, tag="A")
                nc.scalar.activation(t_S[:], ps_ptd[:], Act.Exp, scale=-1.0)
                nc.scalar.activation(t_A[:], ps_ptd[:], Act.Exp, scale=1.0)
                nc.vector.tensor_mul(
                    t_A[:].rearrange("p (k t) -> p k t", k=NT),
                    t_A[:].rearrange("p (k t) -> p k t", k=NT),
                    t_aux[:, 8:16].unsqueeze(1).broadcast_to([128, NT, T]),
                )

                t_rs = cons.tile([128, NT], f32, tag="rs")  # rowsum
                nc.vector.memset(t_rs[:], 0.0)  # HW accum_out accumulates
                t_rcp = cons.tile([128, NT], f32, tag="rcp")  # 1/rowsum

                # ---- k-loop: dist matmul + exp + FK2 + B~ ----
                g_tiles = []
                b_tiles = []
                for k in range(NT):
                    ps_d = psd.tile([128, N], f32, tag="dist")
                    nc.tensor.matmul(
                        ps_d[:, 0:512],
                        t_dab[0:4, k * 128 : (k + 1) * 128],
                        t_dab[0:4, 1032:1544],
                        start=True,
                        stop=True,
                    )
                    nc.tensor.matmul(
                        ps_d[:, 512:1024],
                        t_dab[0:4, k * 128 : (k + 1) * 128],
                        t_dab[0:4, 1544:2056],
                        start=True,
                        stop=True,
                    )
                    t_g = gpool.tile([128, N], f32r, tag="g")
                    nc.scalar.activation(
                        t_g[:],
                        ps_d[:],
                        Act.Exp,
                        bias=t_aux[:, k : k + 1],
                        scale=1.0,
                        accum_out=t_rs[:, k : k + 1],
                    )
                    g_tiles.append(t_g)

                    nc.vector.reciprocal(t_rcp[:, k : k + 1], t_rs[:, k : k + 1])
                    ps_f = psf.tile([128, J], f32, tag="f")
                    nc.tensor.matmul(
                        ps_f[:],
                        t_pk[:, k * 128 : (k + 1) * 128],
                        t_pk[:, 1024:1536],
                        start=True,
                        stop=True,
                    )
                    # B~[p,(t,o)] = (S[p,t] * rcp[p]) * FK2[p,(t,o)]
                    t_b = bpool.tile([128, J], f32r, tag="b")
                    nc.vector.scalar_tensor_tensor(
                        t_b[:].rearrange("p (t o) -> p t o", t=T),
                        t_S[:, k * T : (k + 1) * T].unsqueeze(2).broadcast_to(
                            [128, T, C]
                        ),
                        t_rcp[:, k : k + 1],
                        ps_f[:].rearrange("p (t o) -> p t o", t=T),
                        op0=Alu.mult,
                        op1=Alu.mult,
                    )
                    b_tiles.append(t_b)

                # warm the sqrt activation table after the exps are done
                t_scr = cons.tile([1, 1], f32, tag="scr")
                nc.scalar.activation(t_scr[:], t_rs[0:1, 0:1], Act.Sqrt)

                # ---- n-loop: y matmuls + t-contraction ----
                # bf16 out tiles: rel tol is 2e-2, bf16 keeps ~4e-3; DVE gets
                # its 2x/4x 16-bit modes for the BN apply and the store halves
                out_all = cons.tile([128, NT * C], bf16, tag="out_all")
                sq_all = cons.tile([128, NT * C], bf16, tag="sq_all")
                for n in range(NT):
                    ps_y = psy.tile([128, J], f32, tag="y")
                    for k in range(NT):
                        nc.tensor.matmul(
                            ps_y[:],
                            g_tiles[k][:, n * 128 : (n + 1) * 128],
                            b_tiles[k][:],
                            start=(k == 0),
                            stop=(k == NT - 1),
                        )
                    t_m = work.tile([128, J], f32, tag="m")
                    # DVE (GPSIMD cannot read PSUM)
                    nc.vector.tensor_mul(
                        t_m[:].rearrange("p (t o) -> p t o", t=T),
                        ps_y[:].rearrange("p (t o) -> p t o", t=T),
                        t_A[:, n * T : (n + 1) * T].unsqueeze(2).broadcast_to(
                            [128, T, C]
                        ),
                    )
                    t_out = out_all[:, n * C : (n + 1) * C]
                    nc.vector.tensor_reduce(
                        t_out,
                        t_m[:].rearrange("p (t o) -> p o t", t=T),
                        axis=X,
                        op=Alu.add,
                    )
                    if n == NT - 1:
                        nc.vector.tensor_mul(
                            sq_all[:, n * C : (n + 1) * C], t_out, t_out
                        )
                    else:
                        nc.scalar.activation(
                            sq_all[:, n * C : (n + 1) * C], t_out, Act.Square
                        )

                # ---- stats: colsum matmuls + n-folds ----
                ps_s1 = psy.tile([1, J], f32, tag="y")
                ps_s2 = psy.tile([1, J], f32, tag="y")
                nc.tensor.matmul(
                    ps_s1[:], t_ones_col[:], out_all[:], start=True, stop=True
                )
                nc.tensor.matmul(
                    ps_s2[:], t_ones_col[:], sq_all[:], start=True, stop=True
                )
                t_stat_sb = cons.tile([1, 2 * C], f32, tag="stat_sb")
                nc.vector.tensor_reduce(
                    t_stat_sb[:, 0:C],
                    ps_s1[:].rearrange("p (n o) -> p o n", n=NT),
                    axis=X,
                    op=Alu.add,
                )
                nc.vector.tensor_reduce(
                    t_stat_sb[:, C : 2 * C],
                    ps_s2[:].rearrange("p (n o) -> p o n", n=NT),
                    axis=X,
                    op=Alu.add,
                )

                # ---- BN stats collective + scale/shift ----
                # coll: "gather" (default) = AllGather + local fold;
                # "reduce" = AllReduce + partition-broadcast DMA;
                # False = fake passthrough (single-core timing sim)
                ib = dram.tile([1, 2 * C], f32, tag="ib")
                nc.sync.dma_start(ib[:, 0:C], t_stat_sb[:, 0:C])
                nc.sync.dma_start(ib[:, C : 2 * C], t_stat_sb[:, C : 2 * C])
                P = 1 if coll == "gather" else 128
                t_sums = cons.tile([P, 2 * C], f32, tag="sums")
                if coll == "gather":
                    ob8 = dram.tile([B, 2 * C], f32, tag="ob8")
                    nc.gpsimd.collective_compute(
                        "AllGather",
                        Alu.bypass,
                        replica_groups=[list(range(B))],
                        ins=[ib.opt()],
                        outs=[ob8.opt()],
                    )
                    t_g8 = cons.tile([1, B * 2 * C], f32, tag="g8")
                    nc.sync.dma_start(
                        t_g8[:],
                        ob8[:].rearrange("r x -> (r x)").unsqueeze(0),
                    )
                    nc.vector.tensor_reduce(
                        t_sums[:],
                        t_g8[:].rearrange("p (r x) -> p x r", r=B),
                        axis=X,
                        op=Alu.add,
                    )
                else:
                    ob = dram.tile([1, 2 * C], f32, tag="ob")
                    if coll:
                        nc.gpsimd.collective_compute(
                            "AllReduce",
                            Alu.add,
                            replica_groups=[list(range(B))],
                            ins=[ib.opt()],
                            outs=[ob.opt()],
                        )
                    else:
                        nc.sync.dma_start(ob[:], ib[:])
                    # partition-broadcast summed stats to all 128 partitions
                    nc.sync.dma_start(
                        t_sums[:], ob[:].broadcast_to([128, 2 * C])
                    )

                # scale = gamma * rsqrt(var + eps), shift = beta - mean*scale
                # with var = (s2 - s1^2/BN)/BN, mean = s1/BN, from raw sums.
                rbn = 1.0 / (B * N)
                s1 = t_sums[:, 0:C]
                s2 = t_sums[:, C : 2 * C]
                t_eps = cons.tile([P, 1], f32, tag="eps")
                nc.vector.memset(t_eps[:], BN_EPS)
                t_w1 = cons.tile([P, C], f32, tag="w1")
                nc.vector.tensor_mul(t_w1[:], s1, s1)  # s1^2
                t_var = cons.tile([P, C], f32, tag="var")
                nc.vector.scalar_tensor_tensor(
                    t_var[:], t_w1[:], -rbn, s2, op0=Alu.mult, op1=Alu.add
                )  # s2 - s1^2/BN
                t_std = cons.tile([P, C], f32, tag="std")
                nc.scalar.activation(
                    t_std[:], t_var[:], Act.Sqrt, bias=t_eps[:], scale=rbn
                )
                t_ssw = cons.tile([P, 2 * C], f32, tag="ssw")
                t_rstd = cons.tile([P, C], f32, tag="rstd")
                nc.vector.reciprocal(t_rstd[:], t_std[:])
                gamma = t_aux[0:P, 16:80]
                beta = t_aux[0:P, 80:144]
                nc.vector.tensor_mul(t_ssw[:, 0:C], t_rstd[:], gamma)  # scale
                nc.vector.tensor_mul(t_w1[:], s1, t_ssw[:, 0:C])  # s1*scale
                nc.vector.scalar_tensor_tensor(
                    t_ssw[:, C : 2 * C], t_w1[:], -rbn, beta,
                    op0=Alu.mult, op1=Alu.add,
                )  # beta - mean*scale
                t_ss = cons.tile([128, 2 * C], bf16, tag="ss")
                if P == 1:
                    # broadcast scale|shift to all partitions via matmul
                    t_ones_row = cons.tile([1, 128], f32, tag="ones_row")
                    nc.vector.memset(t_ones_row[:], 1.0)
                    ps_bc = psy.tile([128, 2 * C], f32, tag="y")
                    nc.tensor.matmul(
                        ps_bc[:], t_ones_row[:], t_ssw[:], start=True,
                        stop=True,
                    )
                    nc.vector.tensor_copy(t_ss[:], ps_bc[:])
                else:
                    nc.vector.tensor_copy(t_ss[:], t_ssw[:])

                # ---- apply BN + ReLU, single contiguous store ----
                t_fin = cons.tile([128, NT * C], bf16, tag="fin")
                out_v = out_all[:].rearrange("p (n o) -> p n o", n=NT)
                fin_v = t_fin[:].rearrange("p (n o) -> p n o", n=NT)
                nc.vector.tensor_mul(
                    fin_v,
                    out_v,
                    t_ss[:, 0:C].unsqueeze(1).broadcast_to([128, NT, C]),
                )
                nc.vector.tensor_add(
                    fin_v,
                    fin_v,
                    t_ss[:, C : 2 * C].unsqueeze(1).broadcast_to([128, NT, C]),
                )
                nc.vector.tensor_scalar_max(t_fin[:], t_fin[:], 0.0)
                nc.sync.dma_start(d_out.ap(), t_fin[:])

    nc.compile()
    return nc


def _prep_inputs(points, translations, functions, k_tensor, gamma, beta):
    """Host-side sharding + layout prep (O(N) work only)."""
    k2 = np.ascontiguousarray(
        np.transpose(k_tensor.astype(np.float32), (1, 2, 0)).reshape(C, J)
    )
    g32 = gamma.astype(np.float32)
    b32 = beta.astype(np.float32)
    in_maps = []
    for i in range(B):
        pts = points[i].astype(np.float32)  # [N, 3]
        sq = (pts * pts).sum(axis=1)  # [N]
        tr = translations[i].astype(np.float32)  # [T, 3]
        tsq = (tr * tr).sum(axis=1)  # [T]

        dab = np.zeros((4, 2056), dtype=np.float32)
        dab[0:3, 0:N] = pts.T
        dab[3, 0:N] = 1.0
        dab[0:3, 1024:1032] = tr.T
        dab[0:3, 1032 : 1032 + N] = pts.T
        dab[3, 1032 : 1032 + N] = -0.5 * sq

        aux = np.empty((128, 144), dtype=np.float32)
        aux[:, 0:8] = (-0.5 * sq).reshape(NT, 128).T
        aux[:, 8:16] = np.exp(-0.5 * tsq)[None, :]
        aux[:, 16:80] = g32[None, :]
        aux[:, 80:144] = b32[None, :]

        pk = np.empty((C, 1536), dtype=ml_dtypes.bfloat16)
        pk[:, 0:1024] = functions[i].astype(np.float32).T
        pk[:, 1024:1536] = k2

        in_maps.append(
            {
                "dab": np.ascontiguousarray(dab),
                "aux": np.ascontiguousarray(aux),
                "pk": np.ascontiguousarray(pk),
            }
        )
    return in_maps


def kernel(points, translations, functions, k_tensor, gamma, beta):
    from concourse import bass_utils

    if "nc" not in _CACHE:
        _CACHE["nc"] = _build()
    nc = _CACHE["nc"]

    in_maps = _prep_inputs(points, translations, functions, k_tensor, gamma, beta)
    res = bass_utils.run_bass_kernel_spmd(nc, in_maps, core_ids=list(range(B)))
    # device layout is [p, (n, o)]; row n*128+p of the real output
    out = np.stack(
        [
            res.results[i]["out"]
            .astype(np.float32)
            .reshape(128, NT, C)
            .transpose(1, 0, 2)
            .reshape(N, C)
            for i in range(B)
        ],
        axis=0,
    )
    return out.astype(np.float32)
